# revision 1
# baseline (speedup 1.0000x reference)
"""Multi-head attention (B=4, S=2048, E=1024, H=16) on 8 trn2 NeuronCores.

Sharding: data-parallel over B (4) x tensor-parallel over H (2 halves of 8
heads). Core c handles batch c//2, head-half c%2. Column-parallel qkv_proj,
row-parallel out_proj; the all-reduce of the two partial outputs per batch is
done on the host during unshard (a sum of two arrays), as is the final
transpose (the device emits out^T to keep DMA writes contiguous).

Device kernel (identical program on all 8 cores, bf16 matmul operands with
fp32 PSUM accumulation), software-pipelined around the ACT engine's exp
stream (the hard floor: 33.5M score elements per core):

  For each of 8 attention spans (head-pair p x 1024-query half), per 128-key
  slot jt: score matmuls (scores^T in PSUM) -> ACT exp (scale folded) into
  bf16 SBUF tiles -> next slot, PV matmuls with the exp'd scores as the
  STATIONARY operand (out ctx[q,hd], M=128-dense, half the moving columns of
  the v-stationary form) accumulating across jt, plus 1-cycle ones-column
  matmuls accumulating the softmax denominators. Span epilogue: reciprocal
  of denominators (DVE), per-partition-scalar normalize, PE-transpose of ctx
  chunks back to [hd, q] for the out-projection.

  The qkv projection (q,k per pair; v per pair), and the out-projection are
  emitted as filler groups inside the attention slots so the tensor engine
  stays busy while exp paces the loop.
"""
import sys

import numpy as np

sys.path.insert(0, "/opt/trn_rl_repo")

import ml_dtypes

import concourse.bacc as bacc
import concourse.mybir as mybir
import concourse.tile as tile
from concourse.bass_utils import run_bass_kernel_spmd

F32 = mybir.dt.float32
F32R = mybir.dt.float32r
BF16 = mybir.dt.bfloat16
EXP = mybir.ActivationFunctionType.Exp
FP8 = mybir.dt.float8e4

B, S, E, H, HD = 4, 2048, 1024, 16, 64
HL = 8            # heads per core
SCALE = 1.0 / np.sqrt(E).astype(np.float32)
NP_BF16 = ml_dtypes.bfloat16


def build_nc():
    nc = bacc.Bacc("TRN2", target_bir_lowering=False, debug=False, num_devices=8)
    xt_d = nc.declare_dram_parameter("xt", [E, S], BF16, isOutput=False)
    wqk_d = nc.declare_dram_parameter("wqk", [E, 1024], BF16, isOutput=False)
    wv_d = nc.declare_dram_parameter("wv", [E, 512], BF16, isOutput=False)
    wo_d = nc.declare_dram_parameter("wo", [512, E], BF16, isOutput=False)
    bqk_d = nc.declare_dram_parameter("bqk", [E, 1], F32, isOutput=False)
    bout_d = nc.declare_dram_parameter("bout", [E, 1], F32, isOutput=False)
    bv_d = nc.declare_dram_parameter("bv", [1, 512], BF16, isOutput=False)
    ones_d = nc.declare_dram_parameter("ones", [1, 128], BF16, isOutput=False)
    id_d = nc.declare_dram_parameter("ident", [128, 128], BF16, isOutput=False)
    out_d = nc.declare_dram_parameter("outT", [E, S], F32, isOutput=True)

    with tile.TileContext(nc) as tc:
      with tc.tile_pool(name="pp", bufs=1) as pp, \
           tc.tile_pool(name="ps", bufs=1, space="PSUM") as ps:
        # ---- persistent SBUF tiles
        x_sb = pp.tile([128, 8, S], BF16)        # x^T   [E-part, kt, tok]
        wqk_sb = pp.tile([128, 8, 1024], BF16)   # W_qk  [E-part, kt, qkdim]
        wv_sb = pp.tile([128, 8, 512], BF16)     # W_v   [E-part, kt, vdim]
        qk_sb = pp.tile([128, 8, S], BF16)       # qk^T: m 0-3 q, 4-7 k
        v1_sb = pp.tile([128, 16, 8, HD], BF16)  # v     [tok-part, jt, h, hd]
        wo_sb = pp.tile([128, 4, E], BF16)       # W_out [d-part, ct, e]
        id_sb = pp.tile([128, 128], BF16)
        ones_sb = pp.tile([128, 1], BF16)        # den matmul rhs
        onesrow_sb = pp.tile([1, 128], BF16)     # bias matmul lhsT
        bvrow_sb = pp.tile([1, 512], BF16)
        bqk_sb = pp.tile([128, 8, 1], F32)
        bout_sb = pp.tile([128, 8, 1], F32)
        ctxT_sb = pp.tile([128, 4, S], BF16)     # ctx^T [d-part, ct, tok]

        # ---- PSUM tags (8 banks exactly):
        # s_e/s_o: scores^T per parity [128,1024] (2+2), pv accum [128,16,64]
        # (2, one start per bank; pending-zero auto-zeroes the other chains'
        # first writes), den accumulators (1, same trick), fill: transient
        # qk/v/p4/transpose groups [128,512] (1)
        pv_ps = ps.tile([128, 16, HD], F32)
        den_ps = ps.tile([128, 16], F32)

        # ---- initial DMAs. x by token-quarters (quarter 0 first: it gates
        # the first qk group) on SP; small bias/constant tiles first on Pool.
        # bqk first: it gates the qk evicts on the lead-in critical path.
        # wqk as full rows (each tile delivers the q- AND k-columns at once);
        # x quarter-1 rides the otherwise-idle ACT HWDGE queue.
        # lead-in loads spread across all three DMA queues so no single
        # stream paces the first projection groups: wqk split Pool/SP,
        # x quarter-0 on ACT, x quarter-1 split SP/Pool
        nc.gpsimd.dma_start(out=bqk_sb, in_=bqk_d[:, :]
                            .rearrange("(m p) o -> p m o", p=128))
        for kt in range(4):
            nc.gpsimd.dma_start(out=wqk_sb[:, kt, :],
                                in_=wqk_d[kt * 128:(kt + 1) * 128, :])
        for kt in range(4, 8):
            nc.sync.dma_start(out=wqk_sb[:, kt, :],
                              in_=wqk_d[kt * 128:(kt + 1) * 128, :])
        for kt in range(8):
            nc.scalar.dma_start(
                out=x_sb[:, kt, 0:512], in_=xt_d[kt * 128:(kt + 1) * 128, 0:512])
        for kt in range(4):
            nc.sync.dma_start(
                out=x_sb[:, kt, 512:1024],
                in_=xt_d[kt * 128:(kt + 1) * 128, 512:1024])
        for kt in range(4, 8):
            nc.gpsimd.dma_start(
                out=x_sb[:, kt, 512:1024],
                in_=xt_d[kt * 128:(kt + 1) * 128, 512:1024])
        for kt in range(8):
            nc.gpsimd.dma_start(out=wv_sb[:, kt, :],
                                in_=wv_d[kt * 128:(kt + 1) * 128, :])
        nc.gpsimd.dma_start(out=ones_sb, in_=ones_d[0:1, 0:1]
                            .to_broadcast([128, 1]))
        nc.gpsimd.dma_start(out=onesrow_sb, in_=ones_d[:, :])
        nc.gpsimd.dma_start(out=bvrow_sb, in_=bv_d[:, :])
        nc.gpsimd.dma_start(out=bout_sb, in_=bout_d[:, :]
                            .rearrange("(m p) o -> p m o", p=128))
        # warm the ACT exp table (load is ~2.7us; keep it off the critical path)
        warm = pp.tile([1, 1], F32)
        nc.scalar.activation(out=warm, in_=bqk_sb[0:1, 0, 0:1], func=EXP)
        nc.vector.memset(pv_ps, 0.0)
        nc.vector.memset(den_ps, 0.0)

        def late_x_loads():
            # SP is idle after the prologue; Pool still drains wv/constants
            for ic in range(2, 4):
                for kt in range(8):
                    nc.sync.dma_start(
                        out=x_sb[:, kt, ic * 512:(ic + 1) * 512],
                        in_=xt_d[kt * 128:(kt + 1) * 128,
                                 ic * 512:(ic + 1) * 512])
            nc.sync.dma_start(out=id_sb, in_=id_d[:, :])
            for ct in range(4):
                nc.sync.dma_start(out=wo_sb[:, ct, :],
                                  in_=wo_d[ct * 128:(ct + 1) * 128, :])

        # ================= filler group emitters =================
        def qk_group(m, ic, tag="fill"):
            """One [128 qkdim x 512 tok] projection chunk -> qk_sb (bf16)."""
            pq = ps.tile([128, 512], F32, name=f"pq{m}_{ic}", tag=tag)
            for kt in range(8):
                nc.tensor.matmul(
                    out=pq, lhsT=wqk_sb[:, kt, m * 128:(m + 1) * 128],
                    rhs=x_sb[:, kt, ic * 512:(ic + 1) * 512],
                    start=(kt == 0), stop=(kt == 7))
            nc.vector.tensor_scalar_add(
                qk_sb[:, m, ic * 512:(ic + 1) * 512], pq, bqk_sb[:, m, 0:1])

        def v_group(jt, p):
            """v chunk [128 tok x 128 vdim] for pair p, key-tile jt."""
            pv = ps.tile([128, 128], F32, name=f"pv{jt}_{p}", tag="fill")
            for kt in range(8):
                nc.tensor.matmul(
                    out=pv, lhsT=x_sb[:, kt, jt * 128:(jt + 1) * 128],
                    rhs=wv_sb[:, kt, p * 128:(p + 1) * 128],
                    start=(kt == 0), stop=False)
            nc.tensor.matmul(
                out=pv, lhsT=onesrow_sb, rhs=bvrow_sb[0:1, p * 128:(p + 1) * 128],
                start=False, stop=True)
            nc.vector.tensor_copy(
                v1_sb[:, jt, 2 * p:2 * p + 2, :],
                pv.rearrange("p (h d) -> p h d", d=HD))

        def p4_group(et, i4, tag="fill"):
            """out^T chunk [128 e x 512 tok]."""
            po = ps.tile([128, 512], F32, name=f"po{et}_{i4}", tag=tag)
            for ct in range(4):
                nc.tensor.matmul(
                    out=po, lhsT=wo_sb[:, ct, et * 128:(et + 1) * 128],
                    rhs=ctxT_sb[:, ct, i4 * 512:(i4 + 1) * 512],
                    start=(ct == 0), stop=(ct == 3))
            ot = pp.tile([128, 512], F32, name=f"ot{et}_{i4}", tag="ot", bufs=4)
            nc.vector.tensor_scalar_add(ot, po, bout_sb[:, et, 0:1])
            nc.sync.dma_start(
                out=out_d[et * 128:(et + 1) * 128, i4 * 512:(i4 + 1) * 512],
                in_=ot)

        p4_acc = pp.tile([128, 16, 512], F32)    # partial out-proj (i4 2,3)

        def p4_part(g, et, i4, tag="fill"):
            """ct 0-1 partial of a late out^T chunk -> p4_acc (+bias)."""
            po = ps.tile([128, 512], F32, name=f"pp{et}_{i4}", tag=tag)
            for ct in range(2):
                nc.tensor.matmul(
                    out=po, lhsT=wo_sb[:, ct, et * 128:(et + 1) * 128],
                    rhs=ctxT_sb[:, ct, i4 * 512:(i4 + 1) * 512],
                    start=(ct == 0), stop=(ct == 1))
            nc.vector.tensor_scalar_add(p4_acc[:, g, :], po, bout_sb[:, et, 0:1])

        def p4_part2(g, et, i4, tag="fill"):
            """ct=2 contribution added into p4_acc."""
            po = ps.tile([128, 512], F32, name=f"pq2{et}_{i4}", tag=tag)
            nc.tensor.matmul(
                out=po, lhsT=wo_sb[:, 2, et * 128:(et + 1) * 128],
                rhs=ctxT_sb[:, 2, i4 * 512:(i4 + 1) * 512],
                start=True, stop=True)
            nc.vector.scalar_tensor_tensor(
                out=p4_acc[:, g, :], in0=po, scalar=1.0, in1=p4_acc[:, g, :],
                op0=mybir.AluOpType.mult, op1=mybir.AluOpType.add)

        def p4_fin(g, et, i4, tag="fill"):
            """ct=3 + accumulated partial -> DRAM."""
            po = ps.tile([128, 512], F32, name=f"pf{et}_{i4}", tag=tag)
            nc.tensor.matmul(
                out=po, lhsT=wo_sb[:, 3, et * 128:(et + 1) * 128],
                rhs=ctxT_sb[:, 3, i4 * 512:(i4 + 1) * 512],
                start=True, stop=True)
            ot = pp.tile([128, 512], F32, name=f"of{et}_{i4}", tag="ot", bufs=4)
            nc.vector.scalar_tensor_tensor(
                out=ot, in0=po, scalar=1.0, in1=p4_acc[:, g, :],
                op0=mybir.AluOpType.mult, op1=mybir.AluOpType.add)
            q_ = (nc.sync, nc.scalar)[g % 2]
            q_.dma_start(
                out=out_d[et * 128:(et + 1) * 128, i4 * 512:(i4 + 1) * 512],
                in_=ot)

        # ================= attention span machinery =================
        # span sp = (pair p, icp) over q-tokens icp*1024..+1024, in order
        # (p0,i0),(p0,i1),(p1,i0),... so pair p+1's qk fillers run during
        # pair p's spans.
        SPAN = [(p, icp) for p in range(4) for icp in range(2)]
        e_tiles = {}   # (parity) -> last exp tile (for PV at lag-1-slot)

        def emit_scores_exp(sp, jt, par):
            p, icp = SPAN[sp]
            po_ = par * 64
            s_t = ps.tile([128, 1024], F32, name=f"s{sp}_{jt}_{par}",
                          tag=f"s{par}")
            pb = par * 64
            for ih in range(2):
                nc.tensor.matmul(
                    out=s_t[:, ih * 512:(ih + 1) * 512],
                    lhsT=qk_sb[pb:pb + 64, 4 + p, jt * 128:(jt + 1) * 128],
                    rhs=qk_sb[pb:pb + 64, p,
                              icp * 1024 + ih * 512:icp * 1024 + (ih + 1) * 512],
                    start=True, stop=True)
            e_t = pp.tile([128, 1024], BF16, name=f"e{sp}_{jt}_{par}",
                          tag=f"e{par}", bufs=7)
            nc.scalar.activation(out=e_t, in_=s_t, func=EXP,
                                 scale=float(SCALE))
            e_tiles[par] = e_t

        def emit_pv(sp, jt, e_pair, par):
            # 16 accumulation chains share 2 psum banks (+1 den bank). The
            # banks are explicitly zeroed (Pool memset) at span start and
            # every chain matmul accumulates (start=False): correct under
            # both per-instruction and lazy-zero-region PSUM semantics.
            p, icp = SPAN[sp]
            e_t = e_pair[par]
            for c in range(8):
                ch = par * 8 + c
                nc.tensor.matmul(
                    out=pv_ps[:, ch, :],
                    lhsT=e_t[:, c * 128:(c + 1) * 128],
                    rhs=v1_sb[:, jt, 2 * p + par, :],
                    start=False, stop=False, skip_group_check=True)
                nc.tensor.matmul(
                    out=den_ps[:, ch:ch + 1],
                    lhsT=e_t[:, c * 128:(c + 1) * 128],
                    rhs=ones_sb,
                    start=False, stop=False, skip_group_check=True)

        # span epilogue pieces, drained a few per slot of the next span
        def epi_half(sp, pvs, par):
            """Evict one pv psum bank, then re-zero it for the next span."""
            nc.vector.tensor_copy(pvs[:, par * 8:(par + 1) * 8, :],
                                  pv_ps[:, par * 8:(par + 1) * 8, :])
            if sp < 7:
                nc.vector.memset(pv_ps[:, par * 8:(par + 1) * 8, :], 0.0)

        def epi_start(sp, pvs):
            """Reciprocal of the span's softmax denominators."""
            rcp = pp.tile([128, 16], F32, name=f"rcp{sp}", tag="rcp", bufs=2)
            scr = pp.tile([128, 16], F32, name=f"scr{sp}", tag="scr", bufs=2)
            nc.vector.reciprocal_approx_accurate(
                out=rcp, in_=den_ps, scratch=scr)
            nc.vector.memset(den_ps, 0.0)
            return rcp

        CH_ORDER = [par * 8 + c for c in range(8) for par in range(2)]

        def epi_chain(sp, i, rcp, pvs):
            """Normalize chain i (c-major order) and PE-transpose into ctxT."""
            p, icp = SPAN[sp]
            ch = CH_ORDER[i]
            par, c = ch // 8, ch % 8
            ctxn = pp.tile([128, HD], BF16, name=f"cn{sp}_{ch}", tag="ctxn",
                           bufs=6)
            nc.gpsimd.tensor_scalar_mul(ctxn, pvs[:, ch, :],
                                        rcp[:, ch:ch + 1])
            if sp == 7:
                # attention psum accumulators are dead by now: 16 independent
                # [64,128]bf16 transpose slots, no rotation stalls in the tail
                tp = pv_ps[0:64, ch, :].bitcast(BF16)
            else:
                tp = ps.tile([64, 128], BF16, name=f"tp{sp}_{ch}", tag="fill")
            nc.tensor.transpose(out=tp, in_=ctxn, identity=id_sb)
            dst = ctxT_sb[par * 64:par * 64 + 64, p,
                          icp * 1024 + c * 128:icp * 1024 + (c + 1) * 128]
            if sp == 7:
                nc.scalar.copy(dst, tp)   # ACT is idle after the last exp
            else:
                nc.vector.tensor_copy(dst, tp)

        # ================= static schedule =================
        # filler lists per span: fn closures emitted into slots
        def fills_for(sp):
            p, icp = SPAN[sp]
            fl = {jt: [] for jt in range(16)}
            if sp == 0:
                # rest of pair0: k ic1..3 (needed from jt=4/8/12) + q ic2/3
                # (needed by span 1), one per slot; v pair0 slot-matched;
                # remaining x/id/wo DMAs right after the k-ic1 fold enqueues
                for i, (m, ic) in enumerate(
                        ((4, 1), (4, 2), (0, 2), (0, 3), (4, 3))):
                    fl[2 * i].append(lambda m=m, ic=ic: qk_group(m, ic))
                fl[0].append(late_x_loads)
                for jt in range(16):
                    fl[jt].append(lambda jt=jt: v_group(jt, 0))
            elif sp == 1:                               # qk pair 1
                for i, (m, ic) in enumerate((m, ic) for m in (1, 5)
                                            for ic in range(4)):
                    fl[2 * i].append(lambda m=m, ic=ic: qk_group(m, ic))
            elif sp == 2:                               # v pair1 + qk pair2 (q)
                for jt in range(16):
                    fl[jt].append(lambda jt=jt: v_group(jt, 1))
                for i, ic in enumerate(range(4)):
                    fl[4 * i + 1].append(lambda ic=ic: qk_group(2, ic))
            elif sp == 3:                               # qk pair2 (k) + pair3(q)
                for i, (m, ic) in enumerate((m, ic) for m in (6, 3)
                                            for ic in range(4)):
                    fl[2 * i].append(lambda m=m, ic=ic: qk_group(m, ic))
            elif sp == 4:                               # v pair2 + qk pair3 (k)
                for jt in range(16):
                    fl[jt].append(lambda jt=jt: v_group(jt, 2))
                for i, ic in enumerate(range(4)):
                    fl[4 * i + 1].append(lambda ic=ic: qk_group(7, ic))
            elif sp == 5:                               # v pair3 (needed from
                # span 6) + late out-proj ct0/1 for i4=2
                g23 = list((et, i4) for et in range(8) for i4 in (2, 3))
                for i, (et, i4) in enumerate(t for t in g23 if t[1] == 2):
                    fl[2 * i].append(
                        lambda g=g23.index((et, i4)), et=et, i4=i4:
                        p4_part(g, et, i4))
                for jt in range(16):
                    fl[jt].append(lambda jt=jt: v_group(jt, 3))
            elif sp == 6:                               # late ct0/1 i4=3 + ct2
                g23 = list((et, i4) for et in range(8) for i4 in (2, 3))
                for i, (et, i4) in enumerate(t for t in g23 if t[1] == 3):
                    fl[i].append(
                        lambda g=g23.index((et, i4)), et=et, i4=i4:
                        p4_part(g, et, i4))
                # span-5 chains drain 3/slot at slot ends: c0-3 ready after
                # slot 2, c4-7 after slot 5
                g23 = list((et, i4) for et in range(8) for i4 in (2, 3))
                for i, (et, i4) in enumerate(t for t in g23 if t[1] == 2):
                    fl[4 + i // 2].append(
                        lambda g=g23.index((et, i4)), et=et, i4=i4:
                        p4_part2(g, et, i4))
                for i, (et, i4) in enumerate(t for t in g23 if t[1] == 3):
                    fl[8 + i // 2].append(
                        lambda g=g23.index((et, i4)), et=et, i4=i4:
                        p4_part2(g, et, i4))
            elif sp == 7:                               # out-proj for icp0
                for i, (et, i4) in enumerate((et, i4) for i4 in (0, 1)
                                             for et in range(8)):
                    fl[(4 if i4 == 0 else 8) + (i % 8) // 2].append(
                        lambda et=et, i4=i4: p4_group(et, i4))
            return fl

        # ---- prologue: q-tile pair0 fully + k-tile pair0 ic0, rotating
        # through the (still unused) score psum banks to dodge WAR stalls
        ptags = ["s0", "s1", "fill"]
        qk_group(0, 0, tag="s0")
        qk_group(0, 1, tag="s1")
        qk_group(4, 0, tag="fill")

        pend_epi = []        # [(sp, rcp, pvs, next chain idx)]
        prev_e = None        # e tiles of previous slot
        prev_sp_jt = None
        p4_late = [(et, i4) for et in range(8) for i4 in (2, 3)]

        def drain_epi(n):
            while n > 0 and pend_epi:
                sp_, rcp_, pvs_, ch_ = pend_epi[0]
                epi_chain(sp_, ch_, rcp_, pvs_)
                if ch_ == 15:
                    pend_epi.pop(0)
                else:
                    pend_epi[0] = (sp_, rcp_, pvs_, ch_ + 1)
                n -= 1

        for sp in range(8):
            fl = fills_for(sp)
            for jt in range(16):
                fillers = fl[jt]
                nf = len(fillers)
                # parity-phased emission: all e-side work + half the fillers
                # run while ACT is busy with the o-side exp of the previous
                # slot, and vice versa (PE is in-order; anything queued
                # behind a blocked instruction stalls with it).
                emit_scores_exp(sp, jt, 0)
                if prev_sp_jt is not None:
                    psp, pjt = prev_sp_jt
                    emit_pv(psp, pjt, prev_e, 0)
                    if pjt == 15:
                        pvs = pp.tile([128, 16, HD], F32, name=f"pvs{psp}",
                                      tag="pvs", bufs=2)
                        epi_half(psp, pvs, 0)
                for f in fillers[:nf // 2]:
                    f()
                emit_scores_exp(sp, jt, 1)
                if prev_sp_jt is not None:
                    psp, pjt = prev_sp_jt
                    emit_pv(psp, pjt, prev_e, 1)
                    if pjt == 15:
                        epi_half(psp, pvs, 1)
                        pend_epi.append((psp, epi_start(psp, pvs), pvs, 0))
                for f in fillers[nf // 2:]:
                    f()
                drain_epi(2 if sp in (0, 2, 4, 6) else 3)
                prev_e = dict(e_tiles)
                prev_sp_jt = (sp, jt)

        # ---- tail: last PV, last epilogue, remaining out-proj (rotating
        # psum tags: nothing else lives in the score banks by now)
        emit_pv(7, 15, prev_e, 0)
        pvs = pp.tile([128, 16, HD], F32, name="pvs7", tag="pvs", bufs=2)
        epi_half(7, pvs, 0)
        emit_pv(7, 15, prev_e, 1)
        epi_half(7, pvs, 1)
        pend_epi.append((7, epi_start(7, pvs), pvs, 0))
        drain_epi(8)        # chains c0-3 both parities -> i4=2 tokens ready
        finals = sorted(enumerate(p4_late), key=lambda t: t[1][1])
        for i, (g, (et, i4)) in enumerate(finals):
            p4_fin(g, et, i4, tag=("s0", "s1")[i % 2])
            drain_epi(2)
        drain_epi(32)

    nc.compile()
    return nc


_NC = None


def _get_nc():
    global _NC
    if _NC is None:
        _NC = build_nc()
    return _NC


def make_in_maps(query, Wqkv, bqkv, Wout, bout):
    query = np.asarray(query, dtype=np.float32)
    Wqkv = np.asarray(Wqkv, dtype=np.float32)
    bqkv = np.asarray(bqkv, dtype=np.float32)
    Wout = np.asarray(Wout, dtype=np.float32)
    bout = np.asarray(bout, dtype=np.float32)

    ident = np.eye(128, dtype=NP_BF16)
    ones = np.ones((1, 128), dtype=NP_BF16)

    in_maps = []
    for c in range(8):
        b, hh = c // 2, c % 2
        heads = np.arange(hh * HL, hh * HL + HL)
        dims = (heads[:, None] * HD + np.arange(HD)[None, :]).reshape(-1)  # [512]
        q_rows, k_rows, v_rows = dims, E + dims, 2 * E + dims

        xt = np.ascontiguousarray(query[b].T.astype(NP_BF16))          # [E, S]
        wqk = np.ascontiguousarray(
            np.concatenate([Wqkv[q_rows].T, Wqkv[k_rows].T], axis=1)
            .astype(NP_BF16))                                          # [E,1024]
        wv = np.ascontiguousarray(Wqkv[v_rows].T.astype(NP_BF16))      # [E, 512]
        wo = np.ascontiguousarray(Wout[:, dims].T.astype(NP_BF16))     # [512, E]
        bqk = np.concatenate([bqkv[q_rows], bqkv[k_rows]]).reshape(E, 1)
        bv = bqkv[v_rows].reshape(1, 512).astype(NP_BF16)
        bo = (bout if hh == 0 else np.zeros_like(bout)).reshape(E, 1)

        in_maps.append({
            "xt": xt, "wqk": wqk, "wv": wv, "wo": wo,
            "bqk": np.ascontiguousarray(bqk), "bout": np.ascontiguousarray(bo),
            "bv": bv, "ones": ones, "ident": ident,
        })
    return in_maps


def gather(results):
    out = np.empty((B, S, E), np.float32)
    for b in range(B):
        acc = results[2 * b]["outT"] + results[2 * b + 1]["outT"]   # [E, S]
        out[b] = acc.T
    return out


def kernel(query, key, value, Wqkv, bqkv, Wout, bout):
    # key/value are unused by the reference module (qkv all from query)
    nc = _get_nc()
    in_maps = make_in_maps(query, Wqkv, bqkv, Wout, bout)
    res = run_bass_kernel_spmd(nc, in_maps, list(range(8)))
    return gather(res.results)

